# revision 21
# baseline (speedup 1.0000x reference)
"""GCN classifier (2x GCNConv + JK-cat + mean-pool + linear) on 8 trn2 NeuronCores.

v6 strategy. Dst-range sharding (each core owns the scatter-add for its
6250-node shard). SWDGE gather economics (measured): per-queue drain is
DESCRIPTOR-rate limited (~8.8ns/desc/queue, 4 queues max; per-unit prep on the
queue's Q7 pair is drain-paced), so the schedule minimizes descriptor count
and keeps all 4 queues fed end-to-end.

  - Layer 1 commutes projection with aggregation: x1 = relu((A_hat x) W1 + b1).
    Cores gather raw x rows (256B) from TWO node-major half-tables (int16
    indices span 25088 rows); two gather units per (dst block, half) keep
    chunk dst-spans tight (~25 cols) and S-stream bytes low.
  - Layer 2 gathers padded x1 rows (64 real feats in 256B) from two
    tile-range half-tables, staged per dst-block as L1 flushes and exchanged
    with TWO AllGathers (after blocks 5 and 12). L2 phase A (half 0) drains
    while AllGather 1 completes; phase B re-adds banked f32 partials and
    flushes.
  - Edge chunks (128 slots, dst-sorted) dedup duplicate keys (multi-hot S
    rows). One dma_gather per unit, round-robin over 4 SWDGE queues; the
    first four units are split small so draining starts during prep.
  - Self loops fold into the projection: DVE scales the feat-major x (host
    input xTin) / x1 (xT from L1 flushes) by the diagonal weight per column,
    and one extra matmul accumulates into the projection psum. No per-window
    ACT+PE diagonal work.
  - Block flushes are emitted one unit late (pipeline smoothing); S-tab
    streams alternate sync/scalar HW-DGE queues; header loads front-run.
  - Tail: per-core final linear ([G,7] partial) then a 1.8KB AllReduce.
"""
import numpy as np
import ml_dtypes

import concourse.bacc as bacc
import concourse.bass as bass
import concourse.mybir as mybir
import concourse.tile as tile
from concourse.bass_utils import run_bass_kernel_spmd

F32 = mybir.dt.float32
BF16 = mybir.dt.bfloat16
I16 = mybir.dt.int16
BF_NP = ml_dtypes.bfloat16

N, E, G = 50000, 800000, 64
D_IN, D_H, D_OUT = 128, 64, 7
NC = 8
SH = N // NC            # 6250 nodes per core
TILES = 49              # ceil(SH / 128)
SHP = TILES * 128       # 6272 padded shard rows
NP_ROWS = NC * SHP      # 50176 padded global rows (x table)
XH = NP_ROWS // 2       # 25088 rows per x half-table
WIN = TILES             # 49 windows of 128 dst nodes
BLK_WINS = 4            # windows per psum block ([*, <=512])
NBLK = (WIN + BLK_WINS - 1) // BLK_WINS  # 13
NQ = 4                  # SWDGE queues
NSTR = 2                # streams (x halves / x1 halves)
SUBU = 2                # gather units per (block, stream)
HEAD = 4                # slots in each of the first HEAD_N unit heads
HEAD_N = 4

# layer-2 x1 halves: tile ranges, staged per L1 block, AllGathered after the
# closing block flushes
TR2 = [(0, 24), (24, 49)]
SEC_ROWS = [(t1 - t0) * 128 for t0, t1 in TR2]      # 3072, 3200
SEC_OFF = [t0 * 128 for t0, _ in TR2]
AG_BLK = [5, 12]
AG_LAG = 6
AG1_L2U = 6   # AllGather 1 triggers after this many layer-2 units

_cache = {}


def _l1_units():
    out = []
    for b in range(NBLK):
        for t in range(NSTR):
            out += [(b, t)] * SUBU
    return out


def _l2_units():
    out = []
    for k in range(NSTR):
        for b in range(NBLK):
            out += [(b, k)] * SUBU
    return out


def _schedule_layer(dst_local, core, stream, key, norm, units_bt,
                    head_split):
    """Greedy 128-slot chunking with per-chunk key dedup; (b,t) multiplicity
    in units_bt splits that group's chunks across its occurrences.
    Returns (meta, idx_w [NC,128,nslot*8] i16, s_tab [NC,128,stot] f32).
    """
    blk = dst_local // (BLK_WINS * 128)

    per = {}
    nch = np.zeros((NBLK, NSTR), np.int64)
    for c in range(NC):
        mc = core == c
        lc_all, sc_all, kc_all, nc_all, bc_all = (
            dst_local[mc], stream[mc], key[mc], norm[mc], blk[mc])
        for b in range(NBLK):
            for t in range(NSTR):
                m = (bc_all == b) & (sc_all == t)
                lcs, kcs, ncs = lc_all[m], kc_all[m], nc_all[m]
                order = np.lexsort((kcs, lcs))
                lcs, kcs, ncs = lcs[order], kcs[order], ncs[order]
                ch_id = np.empty(len(lcs), np.int64)
                sl_id = np.empty(len(lcs), np.int64)
                cur, ck = {}, 0
                for i, kk in enumerate(kcs):
                    s = cur.get(kk)
                    if s is None:
                        if len(cur) == 128:
                            cur, ck = {}, ck + 1
                        s = len(cur)
                        cur[kk] = s
                    ch_id[i] = ck
                    sl_id[i] = s
                nchunks = (ck + 1) if len(lcs) else 0
                per[(c, b, t)] = (lcs, kcs, ncs, ch_id, sl_id, nchunks)
                nch[b, t] = max(nch[b, t], nchunks)

    from collections import Counter
    occ = Counter(units_bt)
    seen = Counter()
    slots = []
    units = []
    for pos, (b, t) in enumerate(units_bt):
        n_tot = int(nch[b, t])
        i = seen[(b, t)]
        seen[(b, t)] += 1
        lo = (n_tot * i) // occ[(b, t)]
        hi = (n_tot * (i + 1)) // occ[(b, t)]
        spans = [(lo, hi)]
        if pos < head_split and hi - lo > HEAD:
            spans = [(lo, lo + HEAD), (lo + HEAD, hi)]
        for lo2, hi2 in spans:
            s0 = len(slots)
            for k in range(lo2, hi2):
                slots.append((b, t, k))
            if len(slots) > s0:
                units.append((b, t, s0, len(slots)))
    nslot = len(slots)
    slot_pos = {s: i for i, s in enumerate(slots)}

    c0s = np.full(nslot, 1 << 30, np.int64)
    c1s = np.full(nslot, -1, np.int64)
    for (c, b, t), (lcs, kcs, ncs, ch, sl, nk) in per.items():
        if not len(lcs):
            continue
        cols = lcs - b * BLK_WINS * 128
        for k in range(nk):
            si = slot_pos[(b, t, k)]
            seg = cols[ch == k]
            c0s[si] = min(c0s[si], seg.min())
            c1s[si] = max(c1s[si], seg.max() + 1)
    c0s = np.where(c1s < 0, 0, c0s)
    c1s = np.maximum(c1s, c0s + 1)
    ms = c1s - c0s
    s_off = np.zeros(nslot + 1, np.int64)
    s_off[1:] = np.cumsum(ms)
    stot = int(s_off[-1])

    idx_flat = np.zeros((NC, nslot * 128), np.int16)  # pad idx 0 (S row = 0)
    s_tab = np.zeros((NC, 128, stot), np.float32)
    for (c, b, t), (lcs, kcs, ncs, ch, sl, nk) in per.items():
        if not len(lcs):
            continue
        cols = lcs - b * BLK_WINS * 128
        si_arr = np.array([slot_pos[(b, t, k)] for k in range(nk)],
                          np.int64)
        sis = si_arr[ch]
        idx_flat[c, sis * 128 + sl] = kcs
        np.add.at(s_tab[c], (sl, s_off[sis] + (cols - c0s[sis])), ncs)

    idx_w = np.zeros((NC, 128, nslot * 8), np.int16)
    for c in range(NC):
        w = idx_flat[c].reshape(-1, 16).T
        idx_w[c] = np.tile(w, (8, 1))

    meta = dict(units=units, ms=ms, s_off=s_off, stot=stot,
                col0=c0s.copy(), nslot=nslot)
    return meta, idx_w, s_tab


def _build(meta1, meta2):
    nc = bacc.Bacc("TRN2", target_bir_lowering=False, debug=False,
                   num_devices=NC, num_swdge_queues=NQ)

    xbf_d = nc.dram_tensor("xbf", [NP_ROWS, D_IN], BF16, kind="ExternalInput")
    xTin_d = nc.dram_tensor("xTin", [D_IN, SHP], BF16, kind="ExternalInput")
    dvalT_d = nc.dram_tensor("dvalT", [D_IN, SHP], BF16,
                             kind="ExternalInput")
    idx1_d = nc.dram_tensor("idx1", [128, meta1["nslot"] * 8], I16,
                            kind="ExternalInput")
    idx2_d = nc.dram_tensor("idx2", [128, meta2["nslot"] * 8], I16,
                            kind="ExternalInput")
    s1_d = nc.dram_tensor("s1", [128, meta1["stot"]], BF16,
                          kind="ExternalInput")
    s2_d = nc.dram_tensor("s2", [128, meta2["stot"]], BF16,
                          kind="ExternalInput")
    spool_d = nc.dram_tensor("spool", [128, TILES, G], BF16,
                             kind="ExternalInput")
    w1_d = nc.dram_tensor("W1", [D_IN, D_H], BF16, kind="ExternalInput")
    w2_d = nc.dram_tensor("W2", [D_H, D_H], BF16, kind="ExternalInput")
    wl_d = nc.dram_tensor("Wlin", [2 * D_H, D_OUT], F32, kind="ExternalInput")
    b1_d = nc.dram_tensor("b1", [D_H, 1], F32, kind="ExternalInput")
    b2_d = nc.dram_tensor("b2", [D_H, 1], F32, kind="ExternalInput")
    bl_d = nc.dram_tensor("blin_t", [G, D_OUT], F32, kind="ExternalInput")
    eye64_d = nc.dram_tensor("eye64", [D_H, D_H], BF16, kind="ExternalInput")
    eye64f_d = nc.dram_tensor("eye64f", [D_H, D_H], F32,
                              kind="ExternalInput")
    out_d = nc.dram_tensor("out", [G, D_OUT], F32, kind="ExternalOutput")

    x1s_loc = [nc.dram_tensor(f"x1s_loc{k}", [SEC_ROWS[k], 2 * D_H], BF16)
               for k in range(NSTR)]
    x1s_full = [nc.dram_tensor(f"x1s_full{k}", [NC * SEC_ROWS[k], 2 * D_H],
                               BF16, addr_space="Shared")
                for k in range(NSTR)]
    fin_loc = nc.dram_tensor("fin_loc", [G, D_OUT], F32)
    fin_full = nc.dram_tensor("fin_full", [G, D_OUT], F32,
                              addr_space="Shared")

    max_u = max(max(u[3] - u[2] for u in m["units"]) for m in (meta1, meta2))
    max_s = max(max(int(m["s_off"][u[3]] - m["s_off"][u[2]])
                    for u in m["units"]) for m in (meta1, meta2))
    blk_w = [min((b + 1) * BLK_WINS, WIN) * 128 - b * BLK_WINS * 128
             for b in range(NBLK)]

    with tile.TileContext(nc) as tc:
        with (
            tc.tile_pool(name="persist", bufs=1) as pp,
            tc.tile_pool(name="msg", bufs=10) as mpool,
            tc.tile_pool(name="stabp", bufs=9) as spool_p,
            tc.tile_pool(name="selfp", bufs=2) as selfp,
            tc.tile_pool(name="aggsb", bufs=2) as aggsb,
            tc.tile_pool(name="psAgg", bufs=4, space="PSUM") as psAgg,
            tc.tile_pool(name="psProj", bufs=1, space="PSUM") as psProj,
            tc.tile_pool(name="psTrans", bufs=1, space="PSUM") as psTrans,
            tc.tile_pool(name="psPool", bufs=1, space="PSUM") as psPool,
        ):
            w1_t = pp.tile([D_IN, D_H], BF16)
            w2_t = pp.tile([D_H, D_H], BF16)
            wl_t = pp.tile([2 * D_H, D_OUT], F32)
            b_t = [pp.tile([D_H, 1], F32, name=f"b{i}", tag=f"b{i}")
                   for i in range(2)]
            bl_t = pp.tile([G, D_OUT], F32)
            eye64_t = pp.tile([D_H, D_H], BF16)
            eye64f_t = pp.tile([D_H, D_H], F32)
            zz_t = pp.tile([128, 512], BF16)
            idx_t = [pp.tile([128, m["nslot"] * 8], I16, name=f"idx{i}",
                             tag=f"idx{i}")
                     for i, m in ((0, meta1), (1, meta2))]
            spool_t = pp.tile([128, TILES, G], BF16)
            xTin_t = pp.tile([D_IN, SHP], BF16)
            dvalT_t = pp.tile([D_IN, SHP], BF16)
            xc_t = pp.tile([128, TILES, 2 * D_H], BF16)
            xT_t = [pp.tile([D_H, SHP], BF16, name=f"xT{i}", tag=f"xT{i}")
                    for i in range(2)]
            aggPart = pp.tile([D_H, NBLK, 512], F32)

            # header loads; first-wave idx slices first so gathers start fast
            n1 = meta1["nslot"] * 8
            u0 = meta1["units"][min(7, len(meta1["units"]) - 1)][3] * 8
            # scalar queue's first DMA completes ~14us sooner than sync's
            nc.scalar.dma_start(idx_t[0][:, 0:u0], idx1_d[:, 0:u0])
            half = u0 + (n1 - u0 + 1) // 2
            nc.sync.dma_start(idx_t[0][:, u0:half], idx1_d[:, u0:half])
            nc.sync.dma_start(idx_t[0][:, half:n1], idx1_d[:, half:n1])
            nc.scalar.dma_start(w1_t[:], w1_d[:])
            nc.scalar.dma_start(eye64_t[:], eye64_d[:])
            nc.scalar.dma_start(b_t[0][:], b1_d[:])
            nc.scalar.dma_start(xTin_t[:], xTin_d[:])
            nc.scalar.dma_start(dvalT_t[:], dvalT_d[:])
            nc.scalar.dma_start(idx_t[1][:], idx2_d[:])
            nc.scalar.dma_start(eye64f_t[:], eye64f_d[:])
            nc.scalar.dma_start(w2_t[:], w2_d[:])
            nc.scalar.dma_start(b_t[1][:], b2_d[:])
            nc.scalar.dma_start(wl_t[:], wl_d[:])
            nc.scalar.dma_start(bl_t[:], bl_d[:])
            nc.scalar.dma_start(spool_t[:], spool_d[:])
            nc.vector.memset(zz_t[:], 0.0)

            qn = [0]
            pending = []   # delayed per-unit closures (flush/bank/AG)

            def run_pending():
                while pending:
                    pending.pop(0)()

            def gather_units(L, meta, tables, on_unit_done, psum_mode):
                units, ms, s_off, col0 = (meta["units"], meta["ms"],
                                          meta["s_off"], meta["col0"])
                s_d = s1_d if L == 0 else s2_d
                feat = D_IN if L == 0 else D_H
                for ui, (b, t, a0, a1) in enumerate(units):
                    todo, pending[:] = pending[:], []
                    nh = a1 - a0
                    mt = mpool.tile([128, max_u, D_IN], BF16, name="msg",
                                    tag="msg")
                    nc.gpsimd.dma_gather(
                        mt[:, 0:nh, :], tables[t],
                        idx_t[L][:, a0 * 8:a1 * 8],
                        nh * 128, nh * 128, D_IN,
                        single_packet=False, queue_num=qn[0] % NQ)
                    qn[0] += 1
                    st_t = spool_p.tile([128, max_s], BF16, name="stab",
                                        tag="stab")
                    u_soff = int(s_off[a0])
                    u_slen = int(s_off[a1] - u_soff)
                    eng = (nc.scalar if (ui % 2 == 1 and (L == 1 or ui >= 12))
                           else nc.sync)
                    eng.dma_start(st_t[:, 0:u_slen],
                                  s_d[:, u_soff:u_soff + u_slen])
                    for fn in todo:
                        fn()
                    ps = psum_mode(b)
                    for si in range(a0, a1):
                        m = int(ms[si])
                        so = int(s_off[si] - u_soff)
                        c0 = int(col0[si])
                        nc.tensor.matmul(
                            ps[0:feat, c0:c0 + m],
                            mt[:, si - a0, 0:feat],
                            st_t[:, so:so + m],
                            start=False, stop=True, skip_group_check=True)
                    on_unit_done(b, t, ui)
                run_pending()

            def flush_block(L, ps, b):
                feat = D_IN if L == 0 else D_H
                bw = blk_w[b]
                w0 = b * BLK_WINS * 128
                agg = aggsb.tile([128, 512], BF16, name="aggT", tag="aggT")
                nc.vector.tensor_copy(agg[0:feat, 0:bw], ps[0:feat, 0:bw])
                # self-loop diagonal: dval-scaled feat-major rows, folded
                # into the projection psum
                sm2 = selfp.tile([128, 512], BF16, name="selfm", tag="selfm")
                xsrc = xTin_t if L == 0 else xT_t[0]
                nc.vector.tensor_mul(sm2[0:feat, 0:bw],
                                     xsrc[0:feat, w0:w0 + bw],
                                     dvalT_t[0:feat, w0:w0 + bw])
                ps2 = psProj.tile([D_H, 512], F32, name="proj", tag="proj")
                wt = w1_t if L == 0 else w2_t
                nc.tensor.matmul(ps2[:, 0:bw], wt[0:feat, :],
                                 agg[0:feat, 0:bw], start=True, stop=False)
                nc.tensor.matmul(ps2[:, 0:bw], wt[0:feat, :],
                                 sm2[0:feat, 0:bw], start=False, stop=True)
                xT = xT_t[L]
                nc.scalar.activation(
                    xT[:, w0:w0 + bw], ps2[:, 0:bw],
                    mybir.ActivationFunctionType.Relu, bias=b_t[L])
                for ti in range(b * BLK_WINS, min((b + 1) * BLK_WINS, WIN)):
                    pst = psTrans.tile([128, D_H], BF16, name="pst",
                                       tag="pst")
                    nc.tensor.transpose(
                        pst[:], xT[:, ti * 128:(ti + 1) * 128], eye64_t[:])
                    nc.vector.tensor_copy(
                        xc_t[:, ti, L * D_H:(L + 1) * D_H], pst[:])
                    if L == 1:
                        nc.tensor.matmul(
                            pool_ps[D_H:2 * D_H, :],
                            xc_t[:, ti, D_H:2 * D_H],
                            spool_t[:, ti, :], start=(ti == 0),
                            stop=(ti == WIN - 1),
                            skip_group_check=True)

            # ---------------- layer 1 ----------------
            l1_tables = [xbf_d[k * XH:(k + 1) * XH, :] for k in range(NSTR)]
            ps_blk = {}
            done_units = [0] * NBLK
            upb = [0] * NBLK
            for (b, t, a0, a1) in meta1["units"]:
                upb[b] += 1
            pend_ag = []

            def l1_psum(b):
                if b not in ps_blk:
                    ps = psAgg.tile([128, 512], F32, name="psb", tag="psb")
                    nc.tensor.matmul(ps[0:D_IN, 0:blk_w[b]], zz_t[:, 0:D_IN],
                                     zz_t[:, 0:blk_w[b]], start=True,
                                     stop=True)
                    ps_blk[b] = ps
                return ps_blk[b]

            def l1_done(b, t, ui):
                done_units[b] += 1
                if done_units[b] == upb[b]:
                    ps = ps_blk.pop(b)

                    def fin(b=b, ps=ps):
                        flush_block(0, ps, b)
                        k = 0 if b <= AG_BLK[0] else 1
                        t0b = b * BLK_WINS
                        t1b = min((b + 1) * BLK_WINS, WIN)
                        r0 = t0b * 128 - SEC_OFF[k]
                        r1 = t1b * 128 - SEC_OFF[k]
                        nc.scalar.dma_start(
                            x1s_loc[k][r0:r1, :].rearrange(
                                "(t p) f -> p t f", p=128),
                            xc_t[:, t0b:t1b, :])
                    pending.append(fin)
                    if b == AG_BLK[0]:
                        pend_ag.append((ui + AG_LAG, 0))
                    elif b == AG_BLK[1]:
                        pend_ag.append((ui + AG_LAG, 1))
                for pos, k in list(pend_ag):
                    if pos == ui:
                        pend_ag.remove((pos, k))

                        def ag(k=k):
                            nc.gpsimd.collective_compute(
                                "AllGather", mybir.AluOpType.bypass,
                                replica_groups=[list(range(NC))],
                                ins=[x1s_loc[k][:]], outs=[x1s_full[k][:]])
                        pending.append(ag)

            gather_units(0, meta1, l1_tables, l1_done, l1_psum)
            # half-1 AllGather is deferred into the layer-2 unit stream so
            # its staging wait never head-of-line-blocks phase-A preps
            deferred_ag = [k for pos, k in pend_ag]
            pend_ag.clear()

            # x1 pool partials on PE while layer-2 gathers drain
            pool_ps = psPool.tile([128, G], F32, name="poolps", tag="poolps",
                                  bufs=1)
            for ti in range(TILES):
                nc.tensor.matmul(pool_ps[0:D_H, :], xc_t[:, ti, 0:D_H],
                                 spool_t[:, ti, :], start=(ti == 0),
                                 stop=(ti == TILES - 1),
                                 skip_group_check=True)

            # ---------------- layer 2 ----------------
            l2_tables = [x1s_full[k][:] for k in range(NSTR)]
            ps_blk2 = {}
            done2 = np.zeros((NBLK, NSTR), np.int64)
            upb2 = np.zeros((NBLK, NSTR), np.int64)
            for (b, t, a0, a1) in meta2["units"]:
                upb2[b, t] += 1

            def l2_psum(b):
                if b not in ps_blk2:
                    ps = psAgg.tile([128, 512], F32, name="psb", tag="psb")
                    bw = blk_w[b]
                    if done2[b, 0] < upb2[b, 0]:      # phase A open
                        nc.tensor.matmul(ps[0:D_H, 0:bw], zz_t[:, 0:D_H],
                                         zz_t[:, 0:bw], start=True,
                                         stop=True)
                    else:                              # phase B: re-add bank
                        nc.tensor.matmul(ps[0:D_H, 0:bw], eye64f_t[:],
                                         aggPart[:, b, 0:bw], start=True,
                                         stop=True)
                    ps_blk2[b] = ps
                return ps_blk2[b]

            def l2_done(b, k, ui):
                if ui == AG1_L2U:
                    for kk in deferred_ag:
                        nc.gpsimd.collective_compute(
                            "AllGather", mybir.AluOpType.bypass,
                            replica_groups=[list(range(NC))],
                            ins=[x1s_loc[kk][:]], outs=[x1s_full[kk][:]])
                    deferred_ag.clear()
                done2[b, k] += 1
                if done2[b, k] == upb2[b, k]:
                    ps = ps_blk2.pop(b)
                    if k == 0:
                        def bank(b=b, ps=ps):
                            nc.vector.tensor_copy(aggPart[:, b, 0:blk_w[b]],
                                                  ps[0:D_H, 0:blk_w[b]])
                        pending.append(bank)
                    else:
                        def fin(b=b, ps=ps):
                            flush_block(1, ps, b)
                        pending.append(fin)

            gather_units(1, meta2, l2_tables, l2_done, l2_psum)

            # ---------------- pool + final linear + tiny AllReduce --------
            pool_sb = pp.tile([128, G], F32)
            nc.scalar.copy(pool_sb[:], pool_ps[:])
            fin_ps = psPool.tile([G, D_OUT], F32, name="fin", tag="fin",
                                 bufs=1)
            nc.tensor.matmul(fin_ps[:], pool_sb[:], wl_t[:])
            fin_sb = pp.tile([G, D_OUT], F32)
            nc.scalar.copy(fin_sb[:], fin_ps[:])
            nc.sync.dma_start(fin_loc[:], fin_sb[:])
            nc.gpsimd.collective_compute(
                "AllReduce", mybir.AluOpType.add,
                replica_groups=[list(range(NC))],
                ins=[fin_loc[:]], outs=[fin_full[:]])
            fin_t = pp.tile([G, D_OUT], F32)
            nc.sync.dma_start(fin_t[:], fin_full[:])
            out_t = pp.tile([G, D_OUT], F32)
            nc.vector.tensor_add(out_t[:], fin_t[:], bl_t[:])
            nc.sync.dma_start(out_d[:], out_t[:])

    nc.compile()
    return nc


def _prep_inputs(x, edge_index, batch, W1, b1, W2, b2, Wlin, blin):
    src = np.asarray(edge_index[0]).astype(np.int64)
    dst = np.asarray(edge_index[1]).astype(np.int64)
    dst_all = np.concatenate([dst, np.arange(N, dtype=np.int64)])
    deg = np.bincount(dst_all, minlength=N).astype(np.float64)
    dinv = 1.0 / np.sqrt(np.maximum(deg, 1e-12))
    batch_np = np.asarray(batch).astype(np.int64)

    keep = src != dst
    srck, dstk = src[keep], dst[keep]
    normk = (dinv[srck] * dinv[dstk]).astype(np.float32)
    core = dstk // SH
    dl = dstk - core * SH

    # layer 1: half tables over raw node ids
    st1 = np.minimum(srck // XH, NSTR - 1)
    key1 = (srck - st1 * XH).astype(np.int16)
    meta1, idx1_w, s1_tab = _schedule_layer(
        dl, core, st1, key1, normk, _l1_units(), head_split=HEAD_N)

    # layer 2: padded tile-range half tables (per-core padded layout)
    sc = srck // SH
    sl = srck - sc * SH
    stile = sl // 128
    st2 = (stile >= TR2[1][0]).astype(np.int64)
    srow = sl - np.array(SEC_OFF)[st2]
    key2 = (sc * np.array(SEC_ROWS)[st2] + srow).astype(np.int16)
    meta2, idx2_w, s2_tab = _schedule_layer(
        dl, core, st2, key2, normk, _l2_units(), head_split=0)

    x = np.asarray(x, np.float32)
    xbf = np.zeros((NP_ROWS, D_IN), BF_NP)
    xbf[0:N] = x.astype(BF_NP)

    # diagonal weight: (# self edges incl. added loop) * dinv^2
    mult = 1.0 + np.bincount(dst[src == dst], minlength=N).astype(np.float64)
    dval = (mult * dinv * dinv).astype(np.float32)

    cnt = np.maximum(np.bincount(batch_np, minlength=G), 1).astype(np.float32)
    r = np.arange(SH)
    s_pool = np.zeros((NC, 128, TILES, G), np.float32)
    for c in range(NC):
        g = batch_np[c * SH:(c + 1) * SH]
        s_pool[c, r % 128, r // 128, g] = 1.0 / cnt[g]

    com = dict(
        xbf=xbf,
        W1=np.asarray(W1, np.float32).astype(BF_NP),
        W2=np.asarray(W2, np.float32).astype(BF_NP),
        Wlin=np.asarray(Wlin, np.float32),
        b1=np.asarray(b1, np.float32).reshape(D_H, 1),
        b2=np.asarray(b2, np.float32).reshape(D_H, 1),
        blin_t=np.tile(np.asarray(blin, np.float32), (G, 1)),
        eye64=np.eye(D_H, dtype=BF_NP),
        eye64f=np.eye(D_H, dtype=np.float32),
    )
    in_maps = []
    for c in range(NC):
        xsh = np.zeros((SHP, D_IN), np.float32)
        xsh[0:SH] = x[c * SH:(c + 1) * SH]
        dv = np.zeros((SHP,), np.float32)
        dv[0:SH] = dval[c * SH:(c + 1) * SH]
        in_maps.append(dict(
            com,
            xTin=np.ascontiguousarray(xsh.T).astype(BF_NP),
            dvalT=np.tile(dv[None, :], (D_IN, 1)).astype(BF_NP),
            idx1=idx1_w[c], idx2=idx2_w[c],
            s1=s1_tab[c].astype(BF_NP), s2=s2_tab[c].astype(BF_NP),
            spool=s_pool[c].astype(BF_NP)))
    return meta1, meta2, in_maps


def kernel(x, edge_index, batch, W1, b1, W2, b2, Wlin, blin, _trace=False):
    meta1, meta2, in_maps = _prep_inputs(x, edge_index, batch, W1, b1, W2,
                                         b2, Wlin, blin)
    key = (meta1["nslot"], meta1["stot"], meta2["nslot"], meta2["stot"],
           tuple(meta1["ms"].tolist()), tuple(meta2["ms"].tolist()))
    if key not in _cache:
        _cache.clear()
        _cache[key] = _build(meta1, meta2)
    nc = _cache[key]
    res = run_bass_kernel_spmd(nc, in_maps, list(range(NC)), trace=_trace)
    out = res.results[0]["out"].astype(np.float32)
    if _trace:
        return out, res.exec_time_ns
    return out
